# revision 36
# baseline (speedup 1.0000x reference)
"""Distributed Trainium2 kernel for single-head causal attention.

Problem: B=4, S=2048, d_model=d_attn=1024, f32 I/O.
  q = x@Wq.T; k = x@Wk.T; v = x@Wv.T
  logits = q@k.T  (causal + padding mask, then /sqrt(d_model))
  out = softmax(logits)@v @ Wo.T + bo

Algebraic fold (whole pre/post-softmax chain is linear single-head):
  logits = x (Wq^T Wk) x^T          -> M := Wq^T@Wk   (host, fp32)
  out    = (P x) (Wo Wv)^T + bo     -> N := (Wo@Wv).T (host, fp32)
so the device never materializes K or V.

Sharding (8 cores, no collectives): core = (batch b, group g), and each
core owns four 256-wide q-blocks chosen so the causal k-tile profile is
the SAME on every core:
  block p on g=0 covers q rows [512p, 512p+256)       (q-tiles {4p, 4p+1})
  block p on g=1 covers q rows [512p+256, 512p+512)   (q-tiles {4p+2, 4p+3})
Block p needs k-tiles [0, 4p+4) -> uniform SPMD profile NKT=(4,8,12,16),
40 k-slots/core vs 48 for the 512-wide split: ~10% less PE work. The <=2
boundary k-tiles per block are handled by host-computed multiplicative
0/1 masks on PT (post-exp); interior tiles need no mask.

The Qn projection runs d-major (contraction chunk outermost) in two
4-d2-tile halves of 8 PSUM banks each, so real matmuls start as soon as
the first 0.5 MB chunk pair (M[d], xtq[d]) lands instead of after the
full 4 MB. A short scratch-matmul warmup ahead of it releases the HAM
clock gate during the first chunk's DMA.

On-device chain (all transposed layouts, no on-device transposes):
  QnT[d2,q] -> ST[k,q] -> PT[k,q] -> UT[d,q] -> YT[m,q]
Softmax denominators accumulate on DVE (f32r), are reduced by a
ones-matmul, inverted (f32r reciprocal), and broadcast back to 128
partitions with a rank-1 PE matmul; the slow [1,q] reciprocal chains
hide under neighbouring PE phases.
"""

import os
import sys

sys.path.insert(0, "/opt/trn_rl_repo")

import numpy as np
import ml_dtypes

import concourse.bass as bass  # noqa: F401  (engine types)
import concourse.mybir as mybir
from concourse.bacc import Bacc
from concourse.tile import TileContext
from concourse.bass_utils import run_bass_kernel_spmd

BF = mybir.dt.bfloat16
F32 = mybir.dt.float32
BF_NP = ml_dtypes.bfloat16

P = 128          # partitions / tile edge
B, S, D = 4, 2048, 1024
DC = D // P      # 8 chunks of the contraction axis
BW = 256         # q-block width (ST/AV moving free dim)
NB = 4           # q-blocks per core (each BW wide -> 1024 q rows/core)
NKT = (4, 8, 12, 16)   # k-tiles per q-block (uniform SPMD profile)
KT_TOT = 16      # total k-tiles (S / P)
SCALE = 1.0 / 32.0  # 1/sqrt(d_model)

_NC_CACHE = None
LAST_RESULT = None  # BassKernelResults of the last run (for test.py)


def _q0(g, p):
    # global q-row start of block p on group g
    return 512 * p + 256 * g


def _build():
    nc = Bacc("TRN2")
    xt = nc.dram_tensor("xt", [D, S], BF, kind="ExternalInput")     # x^T
    xtq = nc.dram_tensor("xtq", [D, NB * BW], BF, kind="ExternalInput")
    xkm = nc.dram_tensor("xkm", [S, D], BF, kind="ExternalInput")   # x row-major
    m_d = nc.dram_tensor("m", [D, D], BF, kind="ExternalInput")     # Wq^T@Wk
    n_d = nc.dram_tensor("n", [D, D], BF, kind="ExternalInput")     # (Wo@Wv)^T
    # bo and masks arrive pre-transposed (partition-major) so their DMAs
    # are contiguous per partition (large-element transfers).
    bo_d = nc.dram_tensor("bo", [P, DC], F32, kind="ExternalInput")
    masks = nc.dram_tensor("masks", [P, KT_TOT, BW], BF, kind="ExternalInput")
    out = nc.dram_tensor("out", [2, D, 2 * BW], BF, kind="ExternalOutput")

    with TileContext(nc) as tc:
        with tc.tile_pool(name="persist", bufs=1) as pp:
            # resident tensors
            qt_s = pp.tile([P, DC, NB * BW], BF, tag="qt")    # QnT [d2, q]
            xkm_s = pp.tile([P, KT_TOT, D], BF, tag="xkm")    # x [k, d]
            n_s = pp.tile([P, DC, D], BF, tag="n")            # N [d, m]
            bo_s = pp.tile([P, DC], F32, tag="bo")
            mask_s = pp.tile([P, KT_TOT, BW], BF, tag="mask")
            ones_c = pp.tile([P, 1], mybir.dt.float32r, tag="ones_c")
            ones_r = pp.tile([1, P], mybir.dt.float32r, tag="ones_r")
            ones_rf = pp.tile([1, P], F32, tag="ones_rf")
            ones_cf = pp.tile([P, 1], F32, tag="ones_cf")
            nc.vector.memset(ones_rf[:], 1.0)
            nc.vector.memset(ones_cf[:], 1.0)
            with nc.allow_low_precision(reason="1.0 is exact in fp22"):
                nc.vector.tensor_copy(ones_r[:], ones_rf[:])
                nc.vector.tensor_copy(ones_c[:], ones_cf[:])

            # x^T chunks stay resident (ST's stationary operand)
            xts = []
            for c in range(DC):
                t = pp.tile([P, S], BF, tag=f"xt{c}")
                xts.append(t)

            with tc.tile_pool(name="xw", bufs=1) as xw:
                # Single sync-queue DMA stream, issue order = consumption
                # order. Qn-A (block pair 0) needs only (M[d], xtq[d][:512])
                # per d-layer, so the xtq chunks are split into halves and
                # the hi halves (Qn-B) follow the full lo set. Then x^T
                # (ST), masks, bias, x row-major (AV), N (Y epilogue).
                xtqs, m_c = [], []
                for c in range(DC):
                    t = xw.tile([P, NB * BW], BF, tag=f"xq{c}", name=f"xq{c}")
                    xtqs.append(t)
                    tm = xw.tile([P, D], BF, tag=f"m{c}", name=f"m{c}")
                    m_c.append(tm)
                    nc.sync.dma_start(
                        tm[:], m_d[c * P:(c + 1) * P, :]
                    )
                    nc.sync.dma_start(
                        t[:, :512], xtq[c * P:(c + 1) * P, :512]
                    )
                for c in range(DC):
                    nc.sync.dma_start(
                        xtqs[c][:, 512:], xtq[c * P:(c + 1) * P, 512:]
                    )
                for c in range(DC):
                    nc.sync.dma_start(xts[c][:], xt[c * P:(c + 1) * P, :])
                nc.sync.dma_start(mask_s[:], masks[:, :, :])
                nc.sync.dma_start(bo_s[:], bo_d[:, :])
                nc.sync.dma_start(xkm_s[:], xkm.rearrange("(c p) d -> p c d", p=P))
                nc.sync.dma_start(n_s[:], n_d.rearrange("(c p) m -> p c m", p=P))

                ps = tc.alloc_tile_pool(name="proj_psum", bufs=1, space="PSUM")

                # PE warmup: scratch matmuls while the first chunk DMAs
                # land, so the HAM clock-gate is released when real Qn
                # work starts (~1.7us in).
                scratch = xw.tile([P, 512], BF, tag="scratch")
                nc.vector.memset(scratch[:], 0.0)
                # preload the ACT Exp table now so ST0's first exp does
                # not pay the 1.3us ACT_TABLE_LOAD mid-stream
                dummy = xw.tile([1, P], BF, tag="dummy")
                nc.scalar.activation(
                    dummy[:], ones_rf[:], mybir.ActivationFunctionType.Exp
                )
                warm_ps = ps.tile([P, 512], F32, tag="qn", bufs=8)
                for _ in range(8):
                    nc.tensor.matmul(
                        warm_ps[:], scratch[:, :P], scratch[:],
                        start=True, stop=True,
                    )

                # ---- Qn projection: QnT[d2,q] = sum_d M[d,d2]^T x_q^T[d,q]
                # Half A (block pair 0) runs d-major so it paces with the
                # chunk-pair DMA stream (it needs only the lo xtq halves).
                # Half B's data is resident by then, so it runs at-major:
                # each group's PSUM->SBUF cast hides under the next
                # group's matmuls instead of bunching at the end.
                accs_qa = {}
                for at in range(DC):
                    accs_qa[at] = ps.tile(
                        [P, 512], F32, tag="qn", bufs=8, name=f"qa{at}"
                    )
                for d in range(DC):
                    for at in range(DC):
                        nc.tensor.matmul(
                            accs_qa[at][:],
                            m_c[d][:, at * P:(at + 1) * P],
                            xtqs[d][:, :512],
                            start=(d == 0), stop=(d == DC - 1),
                        )
                for at in range(DC):
                    nc.vector.tensor_copy(
                        qt_s[:, at, :512], accs_qa[at][:]
                    )
                for at in range(DC):
                    acc = ps.tile([P, 512], F32, tag="qn", bufs=8,
                                  name=f"qb{at}")
                    for d in range(DC):
                        nc.tensor.matmul(
                            acc[:],
                            m_c[d][:, at * P:(at + 1) * P],
                            xtqs[d][:, 512:],
                            start=(d == 0), stop=(d == DC - 1),
                        )
                    nc.vector.tensor_copy(qt_s[:, at, 512:], acc[:])
                ps.release()

            # ---- attention ----
            with (
                tc.tile_pool(name="attn", bufs=1) as ap,
                tc.tile_pool(name="attn_psum", bufs=1, space="PSUM") as ps,
            ):
                pts = [[] for _ in range(NB)]
                accs = []
                for p_ in range(NB):
                    acc = ap.tile([P, BW], mybir.dt.float32r, tag="acc",
                                  bufs=NB, name=f"acc{p_}")
                    accs.append(acc)

                def st_tile(p_, kt):
                    # ST[k,q] = x (M x_q^T): lhsT = xt[d2, k-tile],
                    # rhs = QnT[d2, block]; then exp, boundary mask,
                    # denominator partial-sum on DVE.
                    q_sl = slice(p_ * BW, (p_ + 1) * BW)
                    st = ps.tile([P, BW], F32, tag="st", bufs=3)
                    for ac in range(DC):
                        nc.tensor.matmul(
                            st[:],
                            xts[ac][:, kt * P:(kt + 1) * P],
                            qt_s[:, ac, q_sl],
                            start=(ac == 0), stop=(ac == DC - 1),
                        )
                    pt = ap.tile([P, BW], BF, tag=f"pt{p_}_{kt}")
                    nc.scalar.activation(
                        pt[:], st[:], mybir.ActivationFunctionType.Exp,
                        scale=SCALE,
                    )
                    # only the last 4 k-slots of each block are boundary
                    # tiles; interior tiles are all-keep under the causal
                    # mask. masks[kt] is block kt//4's tile (disjoint).
                    if kt >= NKT[p_] - 4:
                        nc.vector.tensor_mul(pt[:], pt[:], mask_s[:, kt, :])
                    with nc.allow_low_precision(
                        reason="fp22 softmax-denominator partials: 1e-4 rel "
                        "err on a positive sum, below the bf16 noise floor"
                    ):
                        if kt == 0:
                            nc.vector.tensor_copy(accs[p_][:], pt[:])
                        else:
                            nc.vector.tensor_add(accs[p_][:], accs[p_][:], pt[:])
                    pts[p_].append(pt)

                def cs_reduce(p_):
                    # colsum[1,q] = ones.T @ acc (partition reduce); the
                    # slow single-lane reciprocal runs on the otherwise
                    # idle ACT engine so it never blocks the DVE FIFO
                    # that the AV normalize-muls need.
                    cs = ps.tile([1, BW], F32, tag="cs", bufs=2)
                    nc.tensor.matmul(
                        cs[:], ones_c[:], accs[p_][:], start=True, stop=True
                    )
                    recip = ap.tile([1, BW], mybir.dt.float32r, tag="recip",
                                    bufs=NB)
                    with nc.allow_low_precision(
                        reason="f32r (fp22) reciprocal row: 6e-5 rel err, "
                        "below this kernel's bf16 noise floor"
                    ):
                        nc.vector.reciprocal(recip[:], cs[:])
                    return recip

                def rb_bcast(recip, on_act=False):
                    # rank-1 PE broadcast of the reciprocal row to [128,q].
                    # The PSUM->SBUF copy can ride the idle ACT engine when
                    # the DVE FIFO is needed elsewhere.
                    rb_ps = ps.tile([P, BW], F32, tag="rb", bufs=1)
                    nc.tensor.matmul(
                        rb_ps[:], ones_r[:], recip[:], start=True, stop=True
                    )
                    rb = ap.tile([P, BW], F32, tag="rb_sb", bufs=2)
                    if on_act:
                        nc.scalar.activation(
                            rb[:], rb_ps[:],
                            mybir.ActivationFunctionType.Identity,
                        )
                    else:
                        nc.vector.tensor_copy(rb[:], rb_ps[:])
                    return rb

                def av_block(p_, rb, upair):
                    # UT[d,q] slice: lhsT = x[k-tile, d-tile], rhs =
                    # PT[k-tile, q]; normalized on the PSUM->SBUF copy,
                    # written into the block-pair U tile for 512-wide Y.
                    # ut shares the "st" PSUM ring (ST is idle by now).
                    col = (p_ % 2) * BW
                    for at in range(DC):
                        ut = ps.tile([P, BW], F32, tag="st", bufs=3,
                                     name=f"ut{p_}_{at}")
                        for i, pt in enumerate(pts[p_]):
                            nc.tensor.matmul(
                                ut[:],
                                xkm_s[:, i, at * P:(at + 1) * P],
                                pt[:],
                                start=(i == 0), stop=(i == len(pts[p_]) - 1),
                            )
                        nc.vector.tensor_mul(
                            upair[:, at, col:col + BW], ut[:], rb[:]
                        )

                def y_pair(pair):
                    # YT[m,q] = N.T-contraction over d of U, + bo;
                    # 512-wide over a block pair. The final pair ships
                    # single-m-tile DMAs at the end to shorten the tail.
                    ytq = ap.tile([P, DC, 2 * BW], BF, tag=f"ytq{pair}",
                                  name=f"ytq{pair}")
                    for mt in range(DC):
                        if pair == 1 and mt == DC - 1:
                            # split the very last m-tile into two 256-wide
                            # chains so the post-last-matmul tail (bias +
                            # out-DMA) is half as long
                            for h in range(2):
                                yth = ps.tile([P, BW], F32, tag="st",
                                              bufs=3, name=f"yt7_{h}")
                                for ac in range(DC):
                                    nc.tensor.matmul(
                                        yth[:],
                                        n_s[:, ac, mt * P:(mt + 1) * P],
                                        upairs[pair][:, ac,
                                                     h * BW:(h + 1) * BW],
                                        start=(ac == 0), stop=(ac == DC - 1),
                                    )
                                nc.scalar.activation(
                                    ytq[:, mt, h * BW:(h + 1) * BW], yth[:],
                                    mybir.ActivationFunctionType.Identity,
                                    bias=bo_s[:, mt:mt + 1],
                                )
                                nc.sync.dma_start(
                                    out[pair, mt * P:(mt + 1) * P,
                                        h * BW:(h + 1) * BW],
                                    ytq[:, mt, h * BW:(h + 1) * BW],
                                )
                            continue
                        yt = ps.tile([P, 2 * BW], F32, tag="yt", bufs=2,
                                     name=f"yt{pair}_{mt}")
                        for ac in range(DC):
                            nc.tensor.matmul(
                                yt[:],
                                n_s[:, ac, mt * P:(mt + 1) * P],
                                upairs[pair][:, ac, :],
                                start=(ac == 0), stop=(ac == DC - 1),
                            )
                        # bias-add on ACT (idle during Y) keeps the DVE
                        # FIFO free for the reciprocal rows hiding here.
                        nc.scalar.activation(
                            ytq[:, mt, :], yt[:],
                            mybir.ActivationFunctionType.Identity,
                            bias=bo_s[:, mt:mt + 1],
                        )
                        if pair == 1 and mt >= 6:
                            nc.sync.dma_start(
                                out[pair, mt * P:(mt + 1) * P, :],
                                ytq[:, mt, :],
                            )
                        elif mt % 2 == 1:
                            lo = mt - 1
                            nc.sync.dma_start(
                                out[pair, lo * P:(lo + 2) * P, :].rearrange(
                                    "(c p) j -> p c j", p=P
                                ),
                                ytq[:, lo:lo + 2, :],
                            )

                # ST phase order 0,1,3,2: each block's 1.7us DVE
                # reciprocal is enqueued where the following PE phase has
                # DVE slack (recip0 under ST3, recip1 under ST2, recip3
                # under AV1, recip2 under Y01) so the strict-FIFO DVE
                # queue never stalls the PE's PSUM-ring recycling.
                recips = [None] * NB
                for p_ in (0, 1, 3, 2):
                    for kt in range(NKT[p_]):
                        st_tile(p_, kt)
                    if p_ == 1:
                        recips[0] = cs_reduce(0)
                    elif p_ == 3:
                        recips[1] = cs_reduce(1)

                # AV + Y, interleaved so output DMA starts mid-kernel.
                upairs = [
                    ap.tile([P, DC, 2 * BW], BF, tag=f"u{i}", name=f"u{i}")
                    for i in range(2)
                ]
                av_block(0, rb_bcast(recips[0]), upairs[0])
                av_block(1, rb_bcast(recips[1]), upairs[0])
                recips[3] = cs_reduce(3)
                recips[2] = cs_reduce(2)
                y_pair(0)
                av_block(2, rb_bcast(recips[2], on_act=True), upairs[1])
                av_block(3, rb_bcast(recips[3], on_act=True), upairs[1])
                y_pair(1)

    nc.compile()
    return nc


def _get_nc():
    global _NC_CACHE
    if _NC_CACHE is None:
        _NC_CACHE = _build()
    return _NC_CACHE


def kernel(x, mask, Wq, Wk, Wv, Wo, bo):
    global LAST_RESULT
    x = np.asarray(x, dtype=np.float32)
    mask = np.asarray(mask, dtype=np.float32)
    Wq = np.asarray(Wq, dtype=np.float32)
    Wk = np.asarray(Wk, dtype=np.float32)
    Wv = np.asarray(Wv, dtype=np.float32)
    Wo = np.asarray(Wo, dtype=np.float32)
    bo = np.asarray(bo, dtype=np.float32)

    m_mat = (Wq.T @ Wk).astype(BF_NP)        # [d1, d2]
    n_mat = (Wo @ Wv).T.astype(BF_NP).copy()  # [d, m]
    bo_r = np.ascontiguousarray(bo.reshape(DC, P).T)   # [P, DC]

    in_maps = []
    for c in range(8):
        b, g = divmod(c, 2)
        xt = x[b].T.astype(BF_NP).copy()                       # [d, s]
        xkm = x[b].astype(BF_NP)                               # [s, d]
        qcols = np.concatenate(
            [np.arange(_q0(g, p_), _q0(g, p_) + BW) for p_ in range(NB)]
        )
        xtq = np.ascontiguousarray(xt[:, qcols])               # [d, 1024]

        m = np.zeros((KT_TOT, P, BW), dtype=np.float32)
        ki = np.arange(P)[:, None]
        qi = np.arange(BW)[None, :]
        for s_ in range(KT_TOT):
            p_ = s_ // 4
            q0 = _q0(g, p_)
            k0 = s_ * P
            mm = ((k0 + ki) <= (q0 + qi)).astype(np.float32)
            mm *= mask[b, k0:k0 + P, None]                     # key padding
            m[s_] = mm
        in_maps.append({
            "xt": xt,
            "xtq": xtq,
            "xkm": xkm,
            "m": m_mat,
            "n": n_mat,
            "bo": bo_r,
            "masks": np.ascontiguousarray(m.transpose(1, 0, 2)).astype(BF_NP),
        })

    nc = _get_nc()
    res = run_bass_kernel_spmd(
        nc, in_maps, core_ids=list(range(8)),
        trace=bool(os.environ.get("ATTN_TRACE")),
    )
    LAST_RESULT = res

    outp = np.empty((B, S, D), dtype=np.float32)
    for c in range(8):
        b, g = divmod(c, 2)
        yt = res.results[c]["out"]                     # [pair, m, 2*BW] bf16
        for p_ in range(NB):
            q0 = _q0(g, p_)
            col = (p_ % 2) * BW
            outp[b, q0:q0 + BW, :] = (
                yt[p_ // 2][:, col:col + BW].T.astype(np.float32)
            )
    return outp
